# revision 30
# baseline (speedup 1.0000x reference)
# Triplet-margin loss kernel for Trainium2 (Bass/Tile), batch-sharded
# across 8 NeuronCores.
#
# reference math (torch F.pairwise_distance semantics):
#   d_ap[b,p] = || anc[b] - pos[b,p] + eps ||_2
#   d_an[b,n] = || anc[b] - neg[b,n] + eps ||_2
#   loss = mean_{b,p,n} max(d_ap[b,p] - d_an[b,n] + margin, 0)
#
# (eps = 1e-6 shifts d^2 by ~3e-8 relative and is dropped; the whole
# pipeline computes on bf16 inputs with fp32 accumulation, which lands
# ~1e-4 relative on the final mean -- far under the 2e-2 gate.)
#
# Engine strategy. DVE reductions (accum ops) run 1 elem/cycle in every
# mode, but plain TensorTensor in bf16 runs 2x_1p (2 elem/cycle). So:
#   - x chunks and anc are DMA'd with an fp32 -> bf16 cast in flight
#     (GpSimd-issued DMAs are the one kind that can cast; HBM traffic
#     is unchanged, SBUF halves, and bf16 enables the 2x subtract).
#   - u = x - anc: one wide bf16 TT per chunk on DVE at 2x (the anc
#     operand rides a stride-0 broadcast AP).
#   - d^2 = sum u^2: ONE 1x reduction per slice -- DVE self-multiply
#     STT w/ fp32 accum or ACT Square w/ accum, split K:(24-K) per tile
#     to balance the engines. (The dot+norm decomposition needs TWO 1x
#     reductions per slice; this needs one plus a half-cost subtract.)
#   - d = sqrt(d^2): one unbiased ACT Sqrt per tile.
#   - pair loss: broadcast (d_ap + margin) - d_an STT on DVE, then a
#     max-0 STT w/ accum on DVE -> per-partition loss sums, split into
#     an early part and a last-chunk part so only ~2 slices of work
#     remain after the final DMA byte; [128, 2*NT] out.
# GpSimd never streams (its SBUF port is shared with DVE and concurrent
# streaming slows DVE 1.5-8x); it only triggers the cast DMAs.

import numpy as np

import concourse.bacc as bacc
import concourse.mybir as mybir
import concourse.tile as tile
from concourse import bass_utils

B, Z = 2048, 1024
NUM_POS, NUM_NEG = 8, 16
NJ = NUM_POS + NUM_NEG
MARGIN = 1.0
N_CORES = 8
BL = B // N_CORES  # 256 rows of anc per core
P = 128
NT = BL // P  # 2 batch-tiles per core

# chunk layout per tile: slice counts per DMA; small final chunks keep
# the post-DMA tail short.
CHUNK_SLICES = [4, 4, 4, 4, 4, 2, 2]
# slices whose d^2 reduction runs on DVE instead of ACT (engine balance;
# slice 22 on DVE lets the two tail-chunk reductions run in parallel)
DVE_NRM = {0, 3, 6, 9, 12, 15, 18, 22}
LATE = 2  # d_an columns computed after the single-slice tail chunks

F32 = mybir.dt.float32
BF16 = mybir.dt.bfloat16
AF = mybir.ActivationFunctionType
OP = mybir.AluOpType


def _emit(tc, nc, anc, pos, neg, out):
    v = nc.vector
    act = nc.scalar
    gp = nc.gpsimd
    pos2 = pos.rearrange("(b j) z -> b (j z)", j=NUM_POS)  # [BL, 8*Z]
    neg2 = neg.rearrange("(b j) z -> b (j z)", j=NUM_NEG)  # [BL, 16*Z]

    starts = np.cumsum([0] + CHUNK_SLICES).tolist()
    assert starts[-1] == NJ

    def chunk_src(ci, b0):
        j0, j1 = starts[ci], starts[ci + 1]
        if j1 <= NUM_POS:
            return pos2[b0 : b0 + P, j0 * Z : j1 * Z]
        assert j0 >= NUM_POS
        return neg2[b0 : b0 + P, (j0 - NUM_POS) * Z : (j1 - NUM_POS) * Z]

    with (
        tc.tile_pool(name="xp", bufs=2 * len(CHUNK_SLICES)) as xp,
        tc.tile_pool(name="up", bufs=len(CHUNK_SLICES)) as up,
        tc.tile_pool(name="apool", bufs=2) as apool,
        tc.tile_pool(name="scp", bufs=1) as scp,
        tc.tile_pool(name="smp", bufs=2) as smp,
        tc.tile_pool(name="opool", bufs=1) as opool,
    ):
        osb = opool.tile([P, 2 * NT], F32, name="osb")
        dve_scr = scp.tile([P, Z], BF16, name="dve_scr")
        act_scr = scp.tile([P, Z], BF16, name="act_scr")
        pair = scp.tile([P, NUM_POS * NUM_NEG], F32, name="pair")
        pair_scr = scp.tile([P, NUM_POS * NUM_NEG], F32, name="pair_scr")
        zero_t = scp.tile([P, NUM_POS * NUM_NEG], F32, name="zero_t")
        v.memset(zero_t[:, :], 0.0)
        for t in range(NT):
            b0 = t * P
            anc_in = apool.tile([P, Z], BF16, name="anc_in")
            nrm = smp.tile([P, NJ], F32, name="nrm")
            dt_ = smp.tile([P, NJ], F32, name="dt_")

            gp.dma_start(anc_in[:, :], anc[b0 : b0 + P, :])

            chunks = []
            for ci in range(len(CHUNK_SLICES)):
                xt = xp.tile([P, CHUNK_SLICES[ci] * Z], BF16, name="xt")
                gp.dma_start(xt[:, :], chunk_src(ci, b0))
                chunks.append(xt)

            # DVE: u = x - anc, one wide 2x bf16 TT per chunk
            us = {}
            for ci, cw in enumerate(CHUNK_SLICES):
                ut = up.tile([P, cw * Z], BF16, name="ut")
                v.tensor_tensor(
                    out=ut[:, :].rearrange("p (c z) -> p c z", c=cw),
                    in0=chunks[ci][:, :].rearrange("p (c z) -> p c z", c=cw),
                    in1=anc_in[:, None, :].broadcast_to([P, cw, Z]),
                    op=OP.subtract,
                )
                for q in range(cw):
                    us[starts[ci] + q] = ut[:, q * Z : (q + 1) * Z]

                # d^2 reductions for this chunk's slices, interleaved so
                # both engines start as soon as each chunk's u exists
                for q in range(cw):
                    jj = starts[ci] + q
                    if jj in DVE_NRM:
                        v.scalar_tensor_tensor(
                            out=dve_scr[:, :],
                            in0=us[jj],
                            scalar=1.0,
                            in1=us[jj],
                            op0=OP.bypass,
                            op1=OP.mult,
                            accum_out=nrm[:, jj : jj + 1],
                        )
                    else:
                        act.activation(
                            act_scr[:, :],
                            us[jj],
                            AF.Square,
                            accum_out=nrm[:, jj : jj + 1],
                        )

            # d = sqrt(d^2), split so only the last chunk's columns gate
            # the post-DMA tail; pair[p,n] = (d_ap_p + margin) - d_an_n
            # and the loss sum = sum relu(pair), likewise split by n.
            NA = NUM_NEG - LATE
            act.activation(dt_[:, 0 : NJ - LATE], nrm[:, 0 : NJ - LATE], AF.Sqrt)
            v.scalar_tensor_tensor(
                out=pair[:, 0 : NUM_POS * NA].rearrange("p (a b) -> p a b", a=NUM_POS),
                in0=dt_[:, 0:NUM_POS, None].broadcast_to([P, NUM_POS, NA]),
                scalar=MARGIN,
                in1=dt_[:, None, NUM_POS : NJ - LATE].broadcast_to([P, NUM_POS, NA]),
                op0=OP.add,
                op1=OP.subtract,
            )
            v.scalar_tensor_tensor(
                out=pair_scr[:, 0 : NUM_POS * NA],
                in0=pair[:, 0 : NUM_POS * NA],
                scalar=0.0,
                in1=zero_t[:, 0 : NUM_POS * NA],
                op0=OP.max,
                op1=OP.add,
                accum_out=osb[:, 2 * t : 2 * t + 1],
            )
            act.activation(dt_[:, NJ - LATE : NJ], nrm[:, NJ - LATE : NJ], AF.Sqrt)
            v.scalar_tensor_tensor(
                out=pair[:, NUM_POS * NA : NUM_POS * NUM_NEG].rearrange(
                    "p (a b) -> p a b", a=NUM_POS
                ),
                in0=dt_[:, 0:NUM_POS, None].broadcast_to([P, NUM_POS, LATE]),
                scalar=MARGIN,
                in1=dt_[:, None, NJ - LATE : NJ].broadcast_to([P, NUM_POS, LATE]),
                op0=OP.add,
                op1=OP.subtract,
            )
            v.scalar_tensor_tensor(
                out=pair_scr[:, NUM_POS * NA : NUM_POS * NUM_NEG],
                in0=pair[:, NUM_POS * NA : NUM_POS * NUM_NEG],
                scalar=0.0,
                in1=zero_t[:, NUM_POS * NA : NUM_POS * NUM_NEG],
                op0=OP.max,
                op1=OP.add,
                accum_out=osb[:, 2 * t + 1 : 2 * t + 2],
            )
        # single_packet: one descriptor/one completion post instead of a
        # 16-queue fan-out the final drain must collect (tiny transfer).
        act.dma_start(out[:, :], osb[:, :], single_packet=True)


_NC_CACHE = None


def build():
    global _NC_CACHE
    if _NC_CACHE is None:
        nc = bacc.Bacc(
            "TRN2", target_bir_lowering=False, debug=False, num_devices=N_CORES
        )
        anc = nc.dram_tensor("anc", (BL, Z), F32, kind="ExternalInput").ap()
        pos = nc.dram_tensor("pos", (BL * NUM_POS, Z), F32, kind="ExternalInput").ap()
        neg = nc.dram_tensor("neg", (BL * NUM_NEG, Z), F32, kind="ExternalInput").ap()
        out = nc.dram_tensor("out", (P, 2 * NT), F32, kind="ExternalOutput").ap()
        with tile.TileContext(nc) as tc:
            _emit(tc, nc, anc, pos, neg, out)
        nc.compile()
        _NC_CACHE = nc
    return _NC_CACHE


def make_in_maps(anc_embedding, pos_embedding, neg_embedding):
    anc_embedding = np.asarray(anc_embedding, dtype=np.float32)
    pos_embedding = np.asarray(pos_embedding, dtype=np.float32)
    neg_embedding = np.asarray(neg_embedding, dtype=np.float32)
    in_maps = []
    for c in range(N_CORES):
        in_maps.append(
            {
                "anc": np.ascontiguousarray(anc_embedding[c * BL : (c + 1) * BL]),
                "pos": np.ascontiguousarray(
                    pos_embedding[c * BL * NUM_POS : (c + 1) * BL * NUM_POS]
                ),
                "neg": np.ascontiguousarray(
                    neg_embedding[c * BL * NUM_NEG : (c + 1) * BL * NUM_NEG]
                ),
            }
        )
    return in_maps


def combine(outs):
    # outs: list of [P, NT] per-core partial sums of relu((d_ap+m) - d_an)
    total = sum(o.astype(np.float64).sum() for o in outs)
    return np.float32(total / (B * NUM_POS * NUM_NEG))


def kernel(anc_embedding, pos_embedding, neg_embedding):
    nc = build()
    in_maps = make_in_maps(anc_embedding, pos_embedding, neg_embedding)
    res = bass_utils.run_bass_kernel_spmd(nc, in_maps, core_ids=list(range(N_CORES)))
    return combine([r["out"] for r in res.results])
